# revision 8
# baseline (speedup 1.0000x reference)
"""Trainium2 Bass kernel for nn_ConcatProductAttention.

Reference computation (B=8, L=2048, D=1024):
    q_s = (q @ W1 + b1)[..., 0]                  # [B, L]
    k_s = (k @ W2 + b2)[..., 0]                  # [B, L]
    attn = softmax_j(where(mask, -inf, q_s[:,i,None] + k_s[:,None,j]))
    attn = where(mask, 0, attn)
    out = attn @ v
    returns (out, attn)

Key algebraic fact: softmax over j of (q_s[i] + k_s[j]) is independent of
q_s[i] (shift invariance), so with e[j] = exp(k_s[j] - C):
    attn[i, j] = (1 - mask[i, j]) * e[j] / S[i]
    S[i]       = sum_j (1 - mask[i, j]) * e[j]
    out[i, :]  = (1 / S[i]) * sum_j (1 - mask[i, j]) * e[j] * v[j, :]
q, W1, b1 are mathematically irrelevant to the outputs.

Per-core (data-parallel over batch, 1 batch per NeuronCore):
    - k_s = rowsum(k * W2_bcast) on VectorE, e = exp(k_s + b2 - 3) on ScalarE
    - ev'[j, 0:D]     = e[j] * v[j, :]   (ScalarE, output rounded to float32r)
      ev'[j, D:D+2]   = e[j]             (S-column, duplicated: f32r needs N>=2)
    - mask~ = (1 - mask) as bf16 (ScalarE), PE-transposed per 128x128 tile to
      get mask~T tiles (contraction over j must sit on the partition dim),
      evacuated PSUM->SBUF with cast to float32r (exact for 0/1 values).
    - P[i, :] = sum_j mask~T[j,i] * ev'[j, :]  accumulated over 16 j-chunks on
      the PE at 1 cycle/row (float32r), giving out*S and S in one pass.
    - out rows = P[:, 0:D] * (1/S) (ScalarE per-partition scale from PSUM)
    - attn rows = mask~ * e_bcast * (1/S)  (VectorE)
"""

import os
import sys

sys.path.insert(0, "/opt/trn_rl_repo")

import numpy as np

B, L, D = 8, 2048, 1024
NCORES = 8
TJ = L // 128  # 16 contraction (j) chunks
TI = L // 128  # 16 output row blocks (i)

_CACHE = {}
LAST_RESULTS = None


def _apply_tile_patches():
    """This container's walrus accepts at most ONE sync-wait per instruction
    (two for EventSemaphore).  Tile's scheduler and its kernel-tail drain can
    emit more; split the excess onto same-engine NOPs (engine queues are
    in-order, so a preceding NOP's wait gates the instruction)."""
    import concourse.mybir as mybir
    import concourse.tile as tile
    import bass_rust

    if getattr(tile.TileContext, "_wait_split_patched", False):
        return

    _orig_add = tile.TileContext._add_instruction

    def _wait_cap(inst):
        return 2 if isinstance(inst, mybir.InstEventSemaphore) else 1

    def _split_add(self, inst):
        si = inst.sync_info
        cap = _wait_cap(inst)
        if si is not None and si.on_wait and len(si.on_wait) > cap:
            waits = list(si.on_wait)
            extra, keep = waits[:-cap], waits[-cap:]
            for w in extra:
                nop = mybir.InstNoOp(
                    name=self.nc.get_next_instruction_name(), ins=[], outs=[]
                )
                nop.engine = inst.engine
                nop.sync_info = bass_rust.SyncInfo(on_wait=[w], on_update=[])
                _orig_add(self, nop)
            inst.sync_info = bass_rust.SyncInfo(
                on_wait=keep, on_update=list(si.on_update or [])
            )
        _orig_add(self, inst)

    def _drain_and_barrier(self, tick_clock, wait_clock):
        nc = self.nc
        gc = tick_clock.global_clock
        allocated = self.sems.allocated()
        for proc_idx, sem in sorted(allocated.items()):
            tick = gc.peek_next(proc_idx) - 1
            if tick > 0:
                mult = 16 if "DMA" in sem.name else 1
                nc.sync.nop().wait_op(sem, tick * mult, "sem-ge")
        nc.sync.drain()
        nc.all_engine_barrier()
        popped = nc._tile_sem_poison_stack.pop()
        assert popped is self._sem_poison
        nc.clear_and_free_semaphores(list(allocated.values()))
        nc.all_engine_barrier()

    tile.TileContext._add_instruction = _split_add
    tile.TileContext._drain_and_barrier = _drain_and_barrier
    tile.TileContext._wait_split_patched = True


def _build_program():
    from contextlib import ExitStack

    import concourse.bass as bass
    import concourse.mybir as mybir
    import concourse.tile as tile
    from concourse.masks import make_identity

    _apply_tile_patches()

    f32 = mybir.dt.float32
    f32r = mybir.dt.float32r
    bf16 = mybir.dt.bfloat16
    u8 = mybir.dt.uint8
    AF = mybir.ActivationFunctionType
    ALU = mybir.AluOpType
    AX = mybir.AxisListType

    nc = bass.Bass()

    k_in = nc.dram_tensor("k", [L, D], f32, kind="ExternalInput")
    v_in = nc.dram_tensor("v", [L, D], f32, kind="ExternalInput")
    m_in = nc.dram_tensor("m", [L, L], u8, kind="ExternalInput")
    w2_in = nc.dram_tensor("w2", [D, 1], f32, kind="ExternalInput")
    b2_in = nc.dram_tensor("b2", [1, 1], f32, kind="ExternalInput")
    out_o = nc.dram_tensor("out", [L, D], f32, kind="ExternalOutput")
    attn_o = nc.dram_tensor("attn", [L, L], f32, kind="ExternalOutput")

    def bcast_ap(ap, parts, free):
        return bass.AP(tensor=ap.tensor, offset=ap.offset, ap=[[0, parts], [1, free]])

    with tile.TileContext(nc) as tc, ExitStack() as ctx:
        const = ctx.enter_context(tc.tile_pool(name="const", bufs=1))

        w2b = const.tile([128, D], f32)
        nc.sync.dma_start(out=w2b, in_=bcast_ap(w2_in[:, :], 128, D))
        b2m3 = const.tile([128, 1], f32)
        nc.sync.dma_start(out=b2m3, in_=bcast_ap(b2_in[:, :], 128, 1))
        nc.vector.tensor_scalar_add(b2m3, b2m3, -3.0)

        ident_bf = const.tile([128, 128], bf16)
        make_identity(nc, ident_bf)
        ident_f32 = const.tile([128, 128], f32)
        make_identity(nc, ident_f32)

        ks_col = const.tile([128, TJ], f32)
        e_col = const.tile([128, TJ], f32)
        r_col = const.tile([128, TI], f32)
        e_bcast = const.tile([128, L], f32)
        evp = const.tile([128, TJ, D + 2], f32r)

        # ---- prologue: k_s, e, ev' -------------------------------------
        with (
            tc.tile_pool(name="kv", bufs=3) as kvp,
            tc.tile_pool(name="prod", bufs=2) as prodp,
        ):
            for t in range(TJ):
                kt = kvp.tile([128, D], f32, tag="kv")
                nc.sync.dma_start(out=kt, in_=k_in[t * 128 : (t + 1) * 128, :])
                pr = prodp.tile([128, D], f32)
                nc.vector.tensor_mul(pr, kt, w2b)
                nc.vector.tensor_reduce(
                    out=ks_col[:, t : t + 1], in_=pr, op=ALU.add, axis=AX.X
                )
            nc.scalar.activation(
                out=e_col, in_=ks_col, func=AF.Exp, bias=b2m3, scale=1.0
            )
            for t in range(TJ):
                vt = kvp.tile([128, D], f32, tag="kv")
                nc.sync.dma_start(out=vt, in_=v_in[t * 128 : (t + 1) * 128, :])
                nc.scalar.mul(
                    out=evp[:, t, 0:D], in_=vt, mul=e_col[:, t : t + 1]
                )
                nc.vector.tensor_copy(
                    out=evp[:, t, D : D + 1], in_=e_col[:, t : t + 1]
                )
                nc.vector.tensor_copy(
                    out=evp[:, t, D + 1 : D + 2], in_=e_col[:, t : t + 1]
                )

        # ---- e_bcast: e_col [128, TJ] -> row [1, L] -> bcast [128, L] --
        with (
            tc.tile_pool(name="pse", bufs=1, space="PSUM") as pse,
            tc.tile_pool(name="ebt", bufs=1) as ebp,
            tc.tile_pool(name="edram", bufs=1, space="DRAM") as edp,
        ):
            et_ps = pse.tile([TJ, 128], f32)
            nc.tensor.transpose(et_ps, e_col, ident_f32)
            et_sb = ebp.tile([TJ, 128], f32)
            nc.vector.tensor_copy(out=et_sb, in_=et_ps)
            scratch = edp.tile([TJ, 128], f32)
            nc.sync.dma_start(out=scratch[:, :], in_=et_sb)
            nc.sync.dma_start(out=e_bcast, in_=bcast_ap(scratch[:, :], 128, L))

        # ---- main loop over row blocks ---------------------------------
        stage = int(os.environ.get("KERNEL_STAGE", "4"))
        n_blocks = int(os.environ.get("KERNEL_NBLOCKS", str(TI if stage >= 4 else (2 if stage >= 2 else 0))))
        with (
            tc.tile_pool(name="mrow", bufs=3) as mrowp,
            tc.tile_pool(name="mnat", bufs=3) as mnatp,
            tc.tile_pool(name="lhsT", bufs=2) as lhsp,
            tc.tile_pool(name="attn", bufs=2) as attnp,
            tc.tile_pool(name="outp", bufs=2) as outp,
            tc.tile_pool(name="srp", bufs=2) as srp,
            tc.tile_pool(name="psP", bufs=2, space="PSUM") as psP,
            tc.tile_pool(name="psT", bufs=2, space="PSUM") as psT,
        ):
            for ib in range(n_blocks):
                rows = slice(ib * 128, (ib + 1) * 128)

                mrow = mrowp.tile([128, L], u8)
                nc.sync.dma_start(out=mrow, in_=m_in[rows, :])
                mnat = mnatp.tile([128, L], bf16)
                nc.scalar.activation(
                    out=mnat, in_=mrow, func=AF.Identity, bias=1.0, scale=-1.0
                )

                lhsT = lhsp.tile([128, TJ, 128], f32r)
                for g in range(2):
                    tp = psT.tile([128, 1024], bf16)
                    for c in range(8):
                        jc = g * 8 + c
                        nc.tensor.transpose(
                            tp[:, c * 128 : (c + 1) * 128],
                            mnat[:, jc * 128 : (jc + 1) * 128],
                            ident_bf,
                        )
                    nc.vector.tensor_copy(
                        out=lhsT[:, g * 8 : (g + 1) * 8, :].rearrange(
                            "p a b -> p (a b)"
                        ),
                        in_=tp,
                    )

                if stage < 3:
                    continue
                P = psP.tile([128, D + 2], f32)
                for jc in range(TJ):
                    lhs = lhsT[:, jc, :]
                    rhs = evp[:, jc, :]
                    st = jc == 0
                    sp = jc == TJ - 1
                    nc.tensor.matmul(
                        P[:, 0:512], lhs, rhs[:, 0:512], start=st, stop=sp
                    )
                    nc.tensor.matmul(
                        P[:, 512:1024], lhs, rhs[:, 512:1024], start=st, stop=sp
                    )
                    nc.tensor.matmul(
                        P[:, 1024:1026], lhs, rhs[:, 1024:1026], start=st, stop=sp
                    )

                s_sb = srp.tile([128, 1], f32)
                nc.vector.tensor_copy(out=s_sb, in_=P[:, 1024:1025])
                nc.vector.reciprocal(out=r_col[:, ib : ib + 1], in_=s_sb)

                o_sb = outp.tile([128, D], f32)
                nc.scalar.mul(out=o_sb, in_=P[:, 0:D], mul=r_col[:, ib : ib + 1])
                nc.sync.dma_start(out=out_o[rows, :], in_=o_sb)

                if stage < 4:
                    continue
                attn_mode = int(os.environ.get("KERNEL_ATTN", "3"))
                a_sb = attnp.tile([128, L], f32)
                nc.vector.tensor_mul(a_sb, mnat, e_bcast)
                if attn_mode >= 2:
                    nc.vector.tensor_scalar_mul(a_sb, a_sb, r_col[:, ib : ib + 1])
                if attn_mode >= 3:
                    nc.sync.dma_start(out=attn_o[rows, :], in_=a_sb)

    return nc


def _get_runner():
    """Build the Bass program once and wrap it in a cached shard_map-jitted
    callable over the 8 NeuronCores (one batch element per core)."""
    if "runner" in _CACHE:
        return _CACHE["runner"]

    import jax
    import concourse.mybir as mybir
    from concourse import bass2jax
    from jax.sharding import Mesh, PartitionSpec
    from jax.experimental.shard_map import shard_map

    bass2jax.install_neuronx_cc_hook()
    nc = _build_program()

    partition_name = (
        nc.partition_id_tensor.name if nc.partition_id_tensor else None
    )
    in_names = []
    out_names = []
    out_avals = []
    for alloc in nc.m.functions[0].allocations:
        if not isinstance(alloc, mybir.MemoryLocationSet):
            continue
        name = alloc.memorylocations[0].name
        if alloc.kind == "ExternalInput":
            if name != partition_name:
                in_names.append(name)
        elif alloc.kind == "ExternalOutput":
            out_names.append(name)
            out_avals.append(
                jax.core.ShapedArray(
                    tuple(alloc.tensor_shape), mybir.dt.np(alloc.dtype)
                )
            )
    n_params = len(in_names)
    all_in_names = in_names + out_names
    if partition_name is not None:
        all_in_names.append(partition_name)

    def _body(*args):
        operands = list(args)
        if partition_name is not None:
            operands.append(bass2jax.partition_id_tensor())
        outs = bass2jax._bass_exec_p.bind(
            *operands,
            out_avals=tuple(out_avals),
            in_names=tuple(all_in_names),
            out_names=tuple(out_names),
            lowering_input_output_aliases=(),
            sim_require_finite=True,
            sim_require_nnan=True,
            nc=nc,
        )
        return tuple(outs)

    devices = jax.devices()[:NCORES]
    mesh = Mesh(np.asarray(devices), ("core",))
    n_args = n_params + len(out_names)
    sharded = jax.jit(
        shard_map(
            _body,
            mesh=mesh,
            in_specs=(PartitionSpec("core"),) * n_args,
            out_specs=(PartitionSpec("core"),) * len(out_names),
            check_rep=False,
        ),
        keep_unused=True,
    )

    runner = {
        "fn": sharded,
        "in_names": in_names,
        "out_names": out_names,
        "out_avals": out_avals,
    }
    _CACHE["runner"] = runner
    return runner


def _prep_inputs(k, v, attn_mask, W2, b2):
    """Concatenated (8*rows, ...) global arrays in runner input order."""
    k = np.ascontiguousarray(np.asarray(k), dtype=np.float32)
    v = np.ascontiguousarray(np.asarray(v), dtype=np.float32)
    m = np.ascontiguousarray(np.asarray(attn_mask)).view(np.uint8)
    w2 = np.ascontiguousarray(np.asarray(W2), dtype=np.float32).reshape(D, 1)
    b2 = np.ascontiguousarray(np.asarray(b2), dtype=np.float32).reshape(1, 1)
    per_core = {
        "k": [k[b] for b in range(B)],
        "v": [v[b] for b in range(B)],
        "m": [m[b] for b in range(B)],
        "w2": [w2] * B,
        "b2": [b2] * B,
    }
    runner = _get_runner()
    concat_in = [
        np.concatenate(per_core[name], axis=0) for name in runner["in_names"]
    ]
    concat_zeros = [
        np.zeros((NCORES * a.shape[0], *a.shape[1:]), a.dtype)
        for a in runner["out_avals"]
    ]
    return concat_in + concat_zeros


def _run(args):
    runner = _get_runner()
    out_arrs = runner["fn"](*args)
    res = {}
    for i, name in enumerate(runner["out_names"]):
        aval = runner["out_avals"][i]
        res[name] = np.asarray(out_arrs[i]).reshape(NCORES, *aval.shape)
    return res


def kernel(q, k, v, attn_mask, W1, b1, W2, b2):
    args = _prep_inputs(k, v, attn_mask, W2, b2)
    res = _run(args)
    return res["out"], res["attn"]


# revision 9
# speedup vs baseline: 180.1568x; 180.1568x over previous
"""Trainium2 Bass kernel for nn_ConcatProductAttention.

Reference computation (B=8, L=2048, D=1024):
    q_s = (q @ W1 + b1)[..., 0]                  # [B, L]
    k_s = (k @ W2 + b2)[..., 0]                  # [B, L]
    attn = softmax_j(where(mask, -inf, q_s[:,i,None] + k_s[:,None,j]))
    attn = where(mask, 0, attn)
    out = attn @ v
    returns (out, attn)

Key algebraic fact: softmax over j of (q_s[i] + k_s[j]) is independent of
q_s[i] (shift invariance), so with e[j] = exp(k_s[j] - C):
    attn[i, j] = (1 - mask[i, j]) * e[j] / S[i]
    S[i]       = sum_j (1 - mask[i, j]) * e[j]
    out[i, :]  = (1 / S[i]) * sum_j (1 - mask[i, j]) * e[j] * v[j, :]
q, W1, b1 are mathematically irrelevant to the outputs.

Per-core (data-parallel over batch, 1 batch per NeuronCore):
    - k_s = rowsum(k * W2_bcast) on VectorE, e = exp(k_s + b2 - 3) on ScalarE
    - ev'[j, 0:D]     = e[j] * v[j, :]   (ScalarE, output rounded to float32r)
      ev'[j, D:D+2]   = e[j]             (S-column, duplicated: f32r needs N>=2)
    - mask~ = (1 - mask) as bf16 (ScalarE), PE-transposed per 128x128 tile to
      get mask~T tiles (contraction over j must sit on the partition dim),
      evacuated PSUM->SBUF with cast to float32r (exact for 0/1 values).
    - P[i, :] = sum_j mask~T[j,i] * ev'[j, :]  accumulated over 16 j-chunks on
      the PE at 1 cycle/row (float32r), giving out*S and S in one pass.
    - out rows = P[:, 0:D] * (1/S) (ScalarE per-partition scale from PSUM)
    - attn rows = mask~ * e_bcast * (1/S)  (VectorE)
"""

import os
import sys

sys.path.insert(0, "/opt/trn_rl_repo")

import numpy as np

B, L, D = 8, 2048, 1024
NCORES = 8
TJ = L // 128  # 16 contraction (j) chunks
TI = L // 128  # 16 output row blocks (i)

_CACHE = {}
LAST_RESULTS = None


def _apply_tile_patches():
    """This container's walrus accepts at most ONE sync-wait per instruction
    (two for EventSemaphore).  Tile's scheduler and its kernel-tail drain can
    emit more; split the excess onto same-engine NOPs (engine queues are
    in-order, so a preceding NOP's wait gates the instruction)."""
    import concourse.mybir as mybir
    import concourse.tile as tile
    import bass_rust

    if getattr(tile.TileContext, "_wait_split_patched", False):
        return

    _orig_add = tile.TileContext._add_instruction

    def _wait_cap(inst):
        return 2 if isinstance(inst, mybir.InstEventSemaphore) else 1

    def _split_add(self, inst):
        si = inst.sync_info
        cap = _wait_cap(inst)
        if si is not None and si.on_wait and len(si.on_wait) > cap:
            waits = list(si.on_wait)
            extra, keep = waits[:-cap], waits[-cap:]
            for w in extra:
                nop = mybir.InstNoOp(
                    name=self.nc.get_next_instruction_name(), ins=[], outs=[]
                )
                nop.engine = inst.engine
                nop.sync_info = bass_rust.SyncInfo(on_wait=[w], on_update=[])
                _orig_add(self, nop)
            inst.sync_info = bass_rust.SyncInfo(
                on_wait=keep, on_update=list(si.on_update or [])
            )
        _orig_add(self, inst)

    def _drain_and_barrier(self, tick_clock, wait_clock):
        nc = self.nc
        gc = tick_clock.global_clock
        allocated = self.sems.allocated()
        for proc_idx, sem in sorted(allocated.items()):
            tick = gc.peek_next(proc_idx) - 1
            if tick > 0:
                mult = 16 if "DMA" in sem.name else 1
                nc.sync.nop().wait_op(sem, tick * mult, "sem-ge")
        nc.sync.drain()
        nc.all_engine_barrier()
        popped = nc._tile_sem_poison_stack.pop()
        assert popped is self._sem_poison
        nc.clear_and_free_semaphores(list(allocated.values()))
        nc.all_engine_barrier()

    tile.TileContext._add_instruction = _split_add
    tile.TileContext._drain_and_barrier = _drain_and_barrier
    tile.TileContext._wait_split_patched = True


def _build_program():
    from contextlib import ExitStack

    import concourse.bass as bass
    import concourse.mybir as mybir
    import concourse.tile as tile
    from concourse.masks import make_identity

    _apply_tile_patches()

    f32 = mybir.dt.float32
    f32r = mybir.dt.float32r
    bf16 = mybir.dt.bfloat16
    u8 = mybir.dt.uint8
    AF = mybir.ActivationFunctionType
    ALU = mybir.AluOpType
    AX = mybir.AxisListType

    nc = bass.Bass()

    k_in = nc.dram_tensor("k", [L, D], f32, kind="ExternalInput")
    v_in = nc.dram_tensor("v", [L, D], f32, kind="ExternalInput")
    m_in = nc.dram_tensor("m", [L, L], u8, kind="ExternalInput")
    w2_in = nc.dram_tensor("w2", [D, 1], f32, kind="ExternalInput")
    b2_in = nc.dram_tensor("b2", [1, 1], f32, kind="ExternalInput")
    out_o = nc.dram_tensor("out", [L, D], f32, kind="ExternalOutput")
    attn_o = nc.dram_tensor("attn", [L, L], f32, kind="ExternalOutput")

    def bcast_ap(ap, parts, free):
        return bass.AP(tensor=ap.tensor, offset=ap.offset, ap=[[0, parts], [1, free]])

    with tile.TileContext(nc) as tc, ExitStack() as ctx:
        const = ctx.enter_context(tc.tile_pool(name="const", bufs=1))

        w2b = const.tile([128, D], f32)
        nc.sync.dma_start(out=w2b, in_=bcast_ap(w2_in[:, :], 128, D))
        b2m3 = const.tile([128, 1], f32)
        nc.sync.dma_start(out=b2m3, in_=bcast_ap(b2_in[:, :], 128, 1))
        nc.vector.tensor_scalar_add(b2m3, b2m3, -3.0)

        ident_bf = const.tile([128, 128], bf16)
        make_identity(nc, ident_bf)
        ident_f32 = const.tile([128, 128], f32)
        make_identity(nc, ident_f32)

        ks_col = const.tile([128, TJ], f32)
        e_col = const.tile([128, TJ], f32)
        r_col = const.tile([128, TI], f32)
        e_bcast = const.tile([128, L], f32)
        evp = const.tile([128, TJ, D + 2], f32r)

        # ---- prologue: k_s, e, ev' -------------------------------------
        with (
            tc.tile_pool(name="kv", bufs=3) as kvp,
            tc.tile_pool(name="prod", bufs=2) as prodp,
        ):
            for t in range(TJ):
                kt = kvp.tile([128, D], f32, tag="kv")
                nc.sync.dma_start(out=kt, in_=k_in[t * 128 : (t + 1) * 128, :])
                pr = prodp.tile([128, D], f32)
                nc.vector.tensor_mul(pr, kt, w2b)
                nc.vector.tensor_reduce(
                    out=ks_col[:, t : t + 1], in_=pr, op=ALU.add, axis=AX.X
                )
            nc.scalar.activation(
                out=e_col, in_=ks_col, func=AF.Exp, bias=b2m3, scale=1.0
            )
            for t in range(TJ):
                vt = kvp.tile([128, D], f32, tag="kv")
                nc.sync.dma_start(out=vt, in_=v_in[t * 128 : (t + 1) * 128, :])
                nc.scalar.mul(
                    out=evp[:, t, 0:D], in_=vt, mul=e_col[:, t : t + 1]
                )
                nc.vector.tensor_copy(
                    out=evp[:, t, D : D + 1], in_=e_col[:, t : t + 1]
                )
                nc.vector.tensor_copy(
                    out=evp[:, t, D + 1 : D + 2], in_=e_col[:, t : t + 1]
                )

        # ---- e_bcast: e_col [128, TJ] -> row [1, L] -> bcast [128, L] --
        with (
            tc.tile_pool(name="pse", bufs=1, space="PSUM") as pse,
            tc.tile_pool(name="ebt", bufs=1) as ebp,
            tc.tile_pool(name="edram", bufs=1, space="DRAM") as edp,
        ):
            et_ps = pse.tile([TJ, 128], f32)
            nc.tensor.transpose(et_ps, e_col, ident_f32)
            et_sb = ebp.tile([TJ, 128], f32)
            nc.vector.tensor_copy(out=et_sb, in_=et_ps)
            scratch = edp.tile([TJ, 128], f32)
            nc.sync.dma_start(out=scratch[:, :], in_=et_sb)
            nc.sync.dma_start(out=e_bcast, in_=bcast_ap(scratch[:, :], 128, L))

        # ---- main loop over row blocks ---------------------------------
        stage = int(os.environ.get("KERNEL_STAGE", "4"))
        n_blocks = int(os.environ.get("KERNEL_NBLOCKS", str(TI if stage >= 4 else (2 if stage >= 2 else 0))))
        with (
            tc.tile_pool(name="mrow", bufs=3) as mrowp,
            tc.tile_pool(name="mnat", bufs=3) as mnatp,
            tc.tile_pool(name="lhsT", bufs=2) as lhsp,
            tc.tile_pool(name="attn", bufs=2) as attnp,
            tc.tile_pool(name="outp", bufs=2) as outp,
            tc.tile_pool(name="srp", bufs=2) as srp,
            tc.tile_pool(name="psP", bufs=2, space="PSUM") as psP,
            tc.tile_pool(name="psT", bufs=2, space="PSUM") as psT,
        ):
            for ib in range(n_blocks):
                rows = slice(ib * 128, (ib + 1) * 128)

                mrow = mrowp.tile([128, L], u8)
                nc.sync.dma_start(out=mrow, in_=m_in[rows, :])
                mnat = mnatp.tile([128, L], bf16)
                nc.scalar.activation(
                    out=mnat, in_=mrow, func=AF.Identity, bias=1.0, scale=-1.0
                )

                lhsT = lhsp.tile([128, TJ, 128], f32r)
                for g in range(2):
                    tp = psT.tile([128, 1024], bf16)
                    for c in range(8):
                        jc = g * 8 + c
                        nc.tensor.transpose(
                            tp[:, c * 128 : (c + 1) * 128],
                            mnat[:, jc * 128 : (jc + 1) * 128],
                            ident_bf,
                        )
                    nc.vector.tensor_copy(
                        out=lhsT[:, g * 8 : (g + 1) * 8, :].rearrange(
                            "p a b -> p (a b)"
                        ),
                        in_=tp,
                    )

                if stage < 3:
                    continue
                P = psP.tile([128, D + 2], f32)
                for jc in range(TJ):
                    lhs = lhsT[:, jc, :]
                    rhs = evp[:, jc, :]
                    st = jc == 0
                    sp = jc == TJ - 1
                    nc.tensor.matmul(
                        P[:, 0:512], lhs, rhs[:, 0:512], start=st, stop=sp
                    )
                    nc.tensor.matmul(
                        P[:, 512:1024], lhs, rhs[:, 512:1024], start=st, stop=sp
                    )
                    nc.tensor.matmul(
                        P[:, 1024:1026], lhs, rhs[:, 1024:1026], start=st, stop=sp
                    )

                s_sb = srp.tile([128, 1], f32)
                nc.vector.tensor_copy(out=s_sb, in_=P[:, 1024:1025])
                nc.vector.reciprocal(out=r_col[:, ib : ib + 1], in_=s_sb)

                o_sb = outp.tile([128, D], f32)
                nc.scalar.mul(out=o_sb, in_=P[:, 0:D], mul=r_col[:, ib : ib + 1])
                nc.sync.dma_start(out=out_o[rows, :], in_=o_sb)

                if stage < 4:
                    continue
                attn_mode = int(os.environ.get("KERNEL_ATTN", "3"))
                a_sb = attnp.tile([128, L], f32)
                nc.vector.tensor_mul(a_sb, mnat, e_bcast)
                if attn_mode >= 2:
                    nc.vector.tensor_scalar_mul(a_sb, a_sb, r_col[:, ib : ib + 1])
                if attn_mode >= 3:
                    nc.sync.dma_start(out=attn_o[rows, :], in_=a_sb)

    return nc


def _get_runner():
    """Build the Bass program once and wrap it in a cached shard_map-jitted
    callable over the 8 NeuronCores (one batch element per core)."""
    if "runner" in _CACHE:
        return _CACHE["runner"]

    import jax
    import concourse.mybir as mybir
    from concourse import bass2jax
    from jax.sharding import Mesh, PartitionSpec
    from jax.experimental.shard_map import shard_map

    bass2jax.install_neuronx_cc_hook()
    nc = _build_program()

    partition_name = (
        nc.partition_id_tensor.name if nc.partition_id_tensor else None
    )
    in_names = []
    out_names = []
    out_avals = []
    for alloc in nc.m.functions[0].allocations:
        if not isinstance(alloc, mybir.MemoryLocationSet):
            continue
        name = alloc.memorylocations[0].name
        if alloc.kind == "ExternalInput":
            if name != partition_name:
                in_names.append(name)
        elif alloc.kind == "ExternalOutput":
            out_names.append(name)
            out_avals.append(
                jax.core.ShapedArray(
                    tuple(alloc.tensor_shape), mybir.dt.np(alloc.dtype)
                )
            )
    n_params = len(in_names)
    all_in_names = in_names + out_names
    if partition_name is not None:
        all_in_names.append(partition_name)

    def _body(*args):
        operands = list(args)
        if partition_name is not None:
            operands.append(bass2jax.partition_id_tensor())
        outs = bass2jax._bass_exec_p.bind(
            *operands,
            out_avals=tuple(out_avals),
            in_names=tuple(all_in_names),
            out_names=tuple(out_names),
            lowering_input_output_aliases=(),
            sim_require_finite=True,
            sim_require_nnan=True,
            nc=nc,
        )
        return tuple(outs)

    devices = jax.devices()[:NCORES]
    mesh = Mesh(np.asarray(devices), ("core",))
    n_args = n_params + len(out_names)
    sharded = jax.jit(
        shard_map(
            _body,
            mesh=mesh,
            in_specs=(PartitionSpec("core"),) * n_args,
            out_specs=(PartitionSpec("core"),) * len(out_names),
            check_rep=False,
        ),
        keep_unused=True,
    )

    runner = {
        "fn": sharded,
        "in_names": in_names,
        "out_names": out_names,
        "out_avals": out_avals,
        "nc": nc,
    }
    _CACHE["runner"] = runner
    return runner


def _prep_inputs(k, v, attn_mask, W2, b2):
    """Concatenated (8*rows, ...) global arrays in runner input order."""
    k = np.ascontiguousarray(np.asarray(k), dtype=np.float32)
    v = np.ascontiguousarray(np.asarray(v), dtype=np.float32)
    m = np.ascontiguousarray(np.asarray(attn_mask)).view(np.uint8)
    w2 = np.ascontiguousarray(np.asarray(W2), dtype=np.float32).reshape(D, 1)
    b2 = np.ascontiguousarray(np.asarray(b2), dtype=np.float32).reshape(1, 1)
    per_core = {
        "k": [k[b] for b in range(B)],
        "v": [v[b] for b in range(B)],
        "m": [m[b] for b in range(B)],
        "w2": [w2] * B,
        "b2": [b2] * B,
    }
    runner = _get_runner()
    concat_in = [
        np.concatenate(per_core[name], axis=0) for name in runner["in_names"]
    ]
    concat_zeros = [
        np.zeros((NCORES * a.shape[0], *a.shape[1:]), a.dtype)
        for a in runner["out_avals"]
    ]
    return concat_in + concat_zeros


def _run(args):
    runner = _get_runner()
    out_arrs = runner["fn"](*args)
    res = {}
    for i, name in enumerate(runner["out_names"]):
        aval = runner["out_avals"][i]
        res[name] = np.asarray(out_arrs[i]).reshape(NCORES, *aval.shape)
    return res


def kernel(q, k, v, attn_mask, W1, b1, W2, b2):
    args = _prep_inputs(k, v, attn_mask, W2, b2)
    res = _run(args)
    return res["out"], res["attn"]
